# revision 4
# baseline (speedup 1.0000x reference)
"""Trainium2 Bass kernel for nn_DemLocGraphEncoder (4-layer GIN + variational heads).

Strategy
--------
The GIN segment-sum aggregation is recast as a dense matmul with a
host-precomputed (I + A)^T adjacency-multiplicity matrix (N=8192, so the
dense form maps perfectly onto the 128x128 TensorEngine; avg degree 32
makes gather/scatter paths no faster and far more complex).

Sharding: nodes are row-sharded 1024/core across 8 cores.  Each layer:
  1. AllGather node features x (node-major) -> x_full  [skipped for layer 0,
     whose input is replicated to every core]
  2. agg^T = x_full^T @ ATshard  on TensorE (feature-major output)
  3. MLP entirely in feature-major form: hT = relu(W1-matmuls + b1),
     xT = relu(W2-matmuls + b2)  (weights replicated, used directly as lhsT)
  4. PE-transpose xT -> node-major x_own, DMA to DRAM for the next AllGather.
Layer 3 folds w2_3 @ {wm,wv} into two fused [2048,128] heads (x3 is never
materialized), then z = mean + var*eps on the VectorEngine.

All matmuls run in fp16 (1 cycle/row on TRN2 PE, fp32 PSUM accumulation;
fp16 chosen over bf16 for its 10-bit mantissa; activations stay < ~1e3 so
no overflow).  Outputs are fp32.
"""

import sys

if "/opt/trn_rl_repo" not in sys.path:
    sys.path.insert(0, "/opt/trn_rl_repo")

import numpy as np

N, E, T, H, O, L = 8192, 262144, 256, 2048, 1024, 128
NC = 8
NS = N // NC          # 1024 nodes per core
P = 128
KT_NODES = N // P     # 64 k-tiles over source nodes
ND = NS // 512        # 2 free-dim tiles over own nodes

_PROGRAM_CACHE = {}


def _build_program():
    import concourse.bass as bass  # noqa: F401
    import concourse.mybir as mybir
    import concourse.tile as tile
    from concourse import bacc
    from concourse.masks import make_identity

    f16 = mybir.dt.float16
    f32 = mybir.dt.float32
    AF = mybir.ActivationFunctionType

    nc = bacc.Bacc("TRN2", target_bir_lowering=False, debug=False, num_devices=NC)

    # ---- I/O ----
    at_d = nc.dram_tensor("at_t", [KT_NODES, ND, P, 512], f16, kind="ExternalInput")
    x0_d = nc.dram_tensor("x0", [KT_NODES, P, T], f16, kind="ExternalInput")
    w_d = {}
    w_d["w1_0"] = nc.dram_tensor("w1_0", [H // P, P, T // P, P], f16, kind="ExternalInput")
    w_d["w2_0"] = nc.dram_tensor("w2_0", [H // P, P, H // P, P], f16, kind="ExternalInput")
    for l in (1, 2):
        w_d[f"w1_{l}"] = nc.dram_tensor(f"w1_{l}", [H // P, P, H // P, P], f16, kind="ExternalInput")
    w_d["w1_3"] = nc.dram_tensor("w1_3", [O // P, P, H // P, P], f16, kind="ExternalInput")
    for l in (1, 2):
        w_d[f"w2_{l}"] = nc.dram_tensor(f"w2_{l}", [H // P, P, H // P, P], f16, kind="ExternalInput")
    whm_d = nc.dram_tensor("whm", [P, O // P, P], f16, kind="ExternalInput")
    whv_d = nc.dram_tensor("whv", [P, O // P, P], f16, kind="ExternalInput")
    b_d = {}
    for l in range(3):
        b_d[f"b1_{l}"] = nc.dram_tensor(f"b1_{l}", [P, H // P], f32, kind="ExternalInput")
    b_d["b1_3"] = nc.dram_tensor("b1_3", [P, O // P], f32, kind="ExternalInput")
    for l in range(3):
        b_d[f"b2_{l}"] = nc.dram_tensor(f"b2_{l}", [P, H // P], f32, kind="ExternalInput")
    bhm_d = nc.dram_tensor("bhm", [P, 1], f32, kind="ExternalInput")
    bhv_d = nc.dram_tensor("bhv", [P, 1], f32, kind="ExternalInput")
    eps_d = nc.dram_tensor("epst", [P, NS], f32, kind="ExternalInput")

    z_d = nc.dram_tensor("zt", [P, NS], f32, kind="ExternalOutput")
    mean_d = nc.dram_tensor("meant", [P, NS], f32, kind="ExternalOutput")
    var_d = nc.dram_tensor("vart", [P, NS], f32, kind="ExternalOutput")

    xown = {l: nc.dram_tensor(f"xown{l}", [NS, H], f16) for l in (1, 2, 3)}
    xg = {l: nc.dram_tensor(f"xg{l}", [N, H], f16, addr_space="Shared") for l in (1, 2, 3)}

    rg = [list(range(NC))]

    with tile.TileContext(nc) as tc:
        with (
            tc.tile_pool(name="const", bufs=1) as const_p,
            tc.tile_pool(name="big", bufs=1) as big_p,
            tc.tile_pool(name="at", bufs=6) as at_p,
            tc.tile_pool(name="xslab", bufs=3) as x_p,
            tc.tile_pool(name="w", bufs=3) as w_p,
            tc.tile_pool(name="xo", bufs=2) as xo_p,
            tc.tile_pool(name="ps", bufs=8, space="PSUM") as ps_p,
        ):
            ident = const_p.tile([P, P], f16, tag="ident")
            make_identity(nc, ident)

            bias_sb = {}
            for name, d in b_d.items():
                bias_sb[name] = const_p.tile(list(d.shape), f32, tag=f"b_{name}", name=f"b_{name}")
                nc.sync.dma_start(bias_sb[name][:], d[:])
            bhm_sb = const_p.tile([P, 1], f32, tag="bhm")
            nc.sync.dma_start(bhm_sb[:], bhm_d[:])
            bhv_sb = const_p.tile([P, 1], f32, tag="bhv")
            nc.sync.dma_start(bhv_sb[:], bhv_d[:])
            eps_sb = const_p.tile([P, NS], f32, tag="eps")
            nc.sync.dma_start(eps_sb[:], eps_d[:])
            whm_sb = const_p.tile([P, O // P, P], f16, tag="whm")
            nc.sync.dma_start(whm_sb[:], whm_d[:])
            whv_sb = const_p.tile([P, O // P, P], f16, tag="whv")
            nc.sync.dma_start(whv_sb[:], whv_d[:])

            def agg(d_in, x_ap_fn, uT):
                """uT[:, mt, n*512:(n+1)*512] = sum_k x[k,m]^T @ AT[k,n]."""
                Mt = d_in // P
                for n in range(ND):
                    for g0 in range(0, Mt, 8):
                        gsz = min(8, Mt - g0)
                        psums = [ps_p.tile([P, 512], f32, tag="mm", name=f"ps{_i}") for _i in range(gsz)]
                        for k in range(KT_NODES):
                            xs = x_p.tile([P, gsz * P], f16, tag="xslab")
                            nc.sync.dma_start(xs[:], x_ap_fn(k, g0 * P, gsz * P))
                            att = at_p.tile([P, 512], f16, tag="at")
                            nc.sync.dma_start(att[:], at_d[k, n])
                            for mi in range(gsz):
                                nc.tensor.matmul(
                                    psums[mi][:],
                                    lhsT=xs[:, mi * P:(mi + 1) * P],
                                    rhs=att[:],
                                    start=(k == 0),
                                    stop=(k == KT_NODES - 1),
                                )
                        for mi in range(gsz):
                            nc.vector.tensor_copy(
                                uT[:, g0 + mi, n * 512:(n + 1) * 512], psums[mi][:]
                            )

            def linear(w_dram, Kt, Mt, rhsT, outT, bias, relu, out_off=0):
                for mt in range(Mt):
                    ws = w_p.tile([P, Kt, P], f16, tag="w")
                    nc.sync.dma_start(ws[:], w_dram[mt])
                    for n in range(ND):
                        p = ps_p.tile([P, 512], f32, tag="mm")
                        for k in range(Kt):
                            nc.tensor.matmul(
                                p[:],
                                lhsT=ws[:, k, :],
                                rhs=rhsT[:, k, n * 512:(n + 1) * 512],
                                start=(k == 0),
                                stop=(k == Kt - 1),
                            )
                        nc.scalar.activation(
                            outT[:, out_off + mt, n * 512:(n + 1) * 512],
                            p[:],
                            AF.Relu if relu else AF.Identity,
                            bias=bias[:, mt:mt + 1],
                        )

            def transpose_store(xT, xown_dram):
                for j in range(NS // P):
                    xo = xo_p.tile([P, H // P, P], f16, tag="xo")
                    for mt in range(H // P):
                        pt = ps_p.tile([P, P], f16, tag="mm")
                        nc.tensor.transpose(pt[:], xT[:, mt, j * P:(j + 1) * P], ident[:])
                        nc.vector.tensor_copy(xo[:, mt, :], pt[:])
                    nc.sync.dma_start(xown_dram[j * P:(j + 1) * P, :], xo[:])

            uT0 = big_p.tile([P, T // P, NS], f16, tag="uT")
            hT = {}
            xT = {}

            # ---- layer 0 ----
            with nc.named_scope("l0_agg"):
                agg(T, lambda k, c0, w: x0_d[k, :, c0:c0 + w], uT0)
            hT[0] = big_p.tile([P, H // P, NS], f16, tag="hT", name="hT0")
            with nc.named_scope("l0_lin1"):
                linear(w_d["w1_0"], T // P, H // P, uT0, hT[0], bias_sb["b1_0"], relu=True)
            xT[0] = big_p.tile([P, H // P, NS], f16, tag="xT", name="xT0")
            with nc.named_scope("l0_lin2"):
                linear(w_d["w2_0"], H // P, H // P, hT[0], xT[0], bias_sb["b2_0"], relu=True)
            with nc.named_scope("l0_tp"):
                transpose_store(xT[0], xown[1])
            with nc.named_scope("ag1"):
                nc.gpsimd.collective_compute(
                    "AllGather", mybir.AluOpType.bypass, replica_groups=rg,
                    ins=[xown[1][:].opt()], outs=[xg[1][:].opt()],
                )

            # ---- layers 1..3 ----
            for l in (1, 2, 3):
                uT = big_p.tile([P, H // P, NS], f16, tag="uT", name=f"uT{l}")
                with nc.named_scope(f"l{l}_agg"):
                    g = xg[l]
                    agg(H, lambda k, c0, w, g=g: g[k * P:(k + 1) * P, c0:c0 + w], uT)
                mt_out = (O if l == 3 else H) // P
                hT[l] = big_p.tile([P, mt_out, NS], f16, tag="hT", name=f"hTl{l}")
                with nc.named_scope(f"l{l}_lin1"):
                    linear(w_d[f"w1_{l}"], H // P, mt_out, uT, hT[l], bias_sb[f"b1_{l}"], relu=True)
                if l < 3:
                    xT[l] = big_p.tile([P, H // P, NS], f16, tag="xT", name=f"xTl{l}")
                    with nc.named_scope(f"l{l}_lin2"):
                        linear(w_d[f"w2_{l}"], H // P, H // P, hT[l], xT[l], bias_sb[f"b2_{l}"], relu=True)
                    with nc.named_scope(f"l{l}_tp"):
                        transpose_store(xT[l], xown[l + 1])
                    with nc.named_scope(f"ag{l + 1}"):
                        nc.gpsimd.collective_compute(
                            "AllGather", mybir.AluOpType.bypass, replica_groups=rg,
                            ins=[xown[l + 1][:].opt()], outs=[xg[l + 1][:].opt()],
                        )

            # ---- fused heads ----
            mean_sb = const_p.tile([P, NS], f32, tag="mean_sb")
            var_sb = const_p.tile([P, NS], f32, tag="var_sb")
            z_sb = const_p.tile([P, NS], f32, tag="z_sb")
            with nc.named_scope("heads"):
                for W_sb, b_sb, o_sb in ((whm_sb, bhm_sb, mean_sb), (whv_sb, bhv_sb, var_sb)):
                    for n in range(ND):
                        p = ps_p.tile([P, 512], f32, tag="mm")
                        for k in range(O // P):
                            nc.tensor.matmul(
                                p[:],
                                lhsT=W_sb[:, k, :],
                                rhs=hT[3][:, k, n * 512:(n + 1) * 512],
                                start=(k == 0),
                                stop=(k == O // P - 1),
                            )
                        nc.scalar.activation(
                            o_sb[:, n * 512:(n + 1) * 512], p[:], AF.Identity,
                            bias=b_sb[:, 0:1],
                        )
                nc.vector.tensor_tensor(z_sb[:], var_sb[:], eps_sb[:], mybir.AluOpType.mult)
                nc.vector.tensor_tensor(z_sb[:], z_sb[:], mean_sb[:], mybir.AluOpType.add)
                nc.sync.dma_start(mean_d[:], mean_sb[:])
                nc.sync.dma_start(var_d[:], var_sb[:])
                nc.sync.dma_start(z_d[:], z_sb[:])

    nc.compile()
    return nc


def _tile_lhsT(w):
    """[K, M] fp16 -> [Mt, 128, Kt, 128]; slab [mt] is SBUF-ready [128p, Kt, 128m]."""
    K, M = w.shape
    Kt, Mt = K // P, M // P
    return np.ascontiguousarray(w.reshape(Kt, P, Mt, P).transpose(2, 1, 0, 3))


def _bias_t(b):
    """[M] fp32 -> [128, Mt] (partition = feature within tile)."""
    return np.ascontiguousarray(b.reshape(-1, P).T).astype(np.float32)


def prepare_inputs(inputs):
    """Host-side preprocessing: adjacency build + layout tiling. Returns in_maps."""
    f16 = np.float16
    eeg_nodes = np.asarray(inputs["eeg_nodes"], np.float32)
    eeg_idx = np.asarray(inputs["eeg_idx"])
    src = eeg_idx[0].astype(np.int64)
    dst = eeg_idx[1].astype(np.int64)

    counts = np.bincount(src * N + dst, minlength=N * N).reshape(N, N)
    AT = counts.astype(np.float32)
    AT[np.arange(N), np.arange(N)] += 1.0  # fold GIN's (1+eps)*x self-term, eps=0
    AT16 = AT.astype(f16)
    del AT, counts

    # Activations explode to ~1.3e5 by layer 3 (> fp16 max).  Since relu is
    # positively homogeneous, scale each of layers 0-2's output by S=1/16
    # (exact power of 2), folded into w2/b2; heads unscale via x S^-3.
    S = np.float32(1.0 / 16.0)
    c = [np.float32(1.0), S, S * S, S * S * S]  # cumulative scale of x_l input

    common = {}
    common["x0"] = np.ascontiguousarray(eeg_nodes.astype(f16).reshape(KT_NODES, P, T))
    for l in range(4):
        common[f"w1_{l}"] = _tile_lhsT(np.asarray(inputs[f"w1_{l}"], np.float32).astype(f16))
        common[f"b1_{l}"] = _bias_t(np.asarray(inputs[f"b1_{l}"], np.float32) * c[l])
    for l in range(3):
        common[f"w2_{l}"] = _tile_lhsT((np.asarray(inputs[f"w2_{l}"], np.float32) * S).astype(f16))
        common[f"b2_{l}"] = _bias_t(np.asarray(inputs[f"b2_{l}"], np.float32) * c[l + 1])

    # fused heads:  mean = h3 @ (w2_3 @ wm) + (b2_3 @ wm + bm); h3 arrives
    # scaled by c[3] so the fused weight is unscaled by 1/c[3].
    w2_3 = np.asarray(inputs["w2_3"], np.float32)
    b2_3 = np.asarray(inputs["b2_3"], np.float32)
    wm = np.asarray(inputs["wm"], np.float32)
    wv = np.asarray(inputs["wv"], np.float32)
    W2m = ((w2_3 @ wm) / c[3]).astype(f16)
    W2v = ((w2_3 @ wv) / c[3]).astype(f16)
    common["whm"] = _tile_lhsT(W2m)[0]
    common["whv"] = _tile_lhsT(W2v)[0]
    common["bhm"] = (b2_3 @ wm + np.asarray(inputs["bm"], np.float32)).reshape(P, 1).astype(np.float32)
    common["bhv"] = (b2_3 @ wv + np.asarray(inputs["bv"], np.float32)).reshape(P, 1).astype(np.float32)

    eps = np.asarray(inputs["eps"], np.float32)
    in_maps = []
    for c in range(NC):
        m = dict(common)
        blk = AT16[:, c * NS:(c + 1) * NS]
        m["at_t"] = np.ascontiguousarray(
            blk.reshape(KT_NODES, P, ND, 512).transpose(0, 2, 1, 3)
        )
        m["epst"] = np.ascontiguousarray(eps[c * NS:(c + 1) * NS, :].T)
        in_maps.append(m)
    return in_maps


def get_program():
    if "nc" not in _PROGRAM_CACHE:
        _PROGRAM_CACHE["nc"] = _build_program()
    return _PROGRAM_CACHE["nc"]


def assemble_outputs(results):
    z = np.empty((N, L), np.float32)
    mean = np.empty((N, L), np.float32)
    var = np.empty((N, L), np.float32)
    for c in range(NC):
        z[c * NS:(c + 1) * NS] = results[c]["zt"].T
        mean[c * NS:(c + 1) * NS] = results[c]["meant"].T
        var[c * NS:(c + 1) * NS] = results[c]["vart"].T
    return z, mean, var


def kernel(**inputs):
    from concourse.bass_utils import run_bass_kernel_spmd

    nc = get_program()
    in_maps = prepare_inputs(inputs)
    res = run_bass_kernel_spmd(nc, in_maps, core_ids=list(range(NC)))
    return assemble_outputs(res.results)


# revision 10
# speedup vs baseline: 1.0743x; 1.0743x over previous
"""Trainium2 Bass kernel for nn_DemLocGraphEncoder (4-layer GIN + variational heads).

Strategy
--------
The GIN segment-sum aggregation is recast as a dense matmul with a
host-precomputed (I + A)^T adjacency-multiplicity matrix (N=8192, so the
dense form maps perfectly onto the 128x128 TensorEngine; avg degree 32
makes gather/scatter paths no faster and far more complex).

Sharding: nodes are row-sharded 1024/core across 8 cores.  Each layer:
  1. AllGather node features x (node-major) -> x_full  [skipped for layer 0,
     whose input is replicated to every core]
  2. agg^T = x_full^T @ ATshard  on TensorE (feature-major output)
  3. MLP entirely in feature-major form: hT = relu(W1-matmuls + b1),
     xT = relu(W2-matmuls + b2)  (weights replicated, used directly as lhsT)
  4. PE-transpose xT -> node-major x_own, DMA to DRAM for the next AllGather.
Layer 3 folds w2_3 @ {wm,wv} into two fused [2048,128] heads (x3 is never
materialized), then z = mean + var*eps on the VectorEngine.

All matmuls run in fp16 (1 cycle/row on TRN2 PE, fp32 PSUM accumulation;
fp16 chosen over bf16 for its 10-bit mantissa; activations stay < ~1e3 so
no overflow).  Outputs are fp32.
"""

import sys

if "/opt/trn_rl_repo" not in sys.path:
    sys.path.insert(0, "/opt/trn_rl_repo")

import numpy as np

N, E, T, H, O, L = 8192, 262144, 256, 2048, 1024, 128
NC = 8
NS = N // NC          # 1024 nodes per core
P = 128
KT_NODES = N // P     # 64 k-tiles over source nodes
ND = NS // 512        # 2 free-dim tiles over own nodes

_PROGRAM_CACHE = {}


def _build_program(collectives=True, opts=None):
    opts = dict(opts or {})
    no_transpose = opts.get("no_transpose", False)   # sim-only: DMA instead of PE transpose
    drain_split = opts.get("drain_split", True)     # alternate agg drains DVE/ACT
    at_bufs = opts.get("at_bufs", 10)
    x_bufs = opts.get("x_bufs", 5)
    w_bufs = opts.get("w_bufs", 4)
    ps_bufs = opts.get("ps_bufs", 8)
    agg_group = opts.get("agg_group", 8)
    import concourse.bass as bass  # noqa: F401
    import concourse.mybir as mybir
    import concourse.tile as tile
    from concourse import bacc
    from concourse.masks import make_identity

    f16 = mybir.dt.float16
    f32 = mybir.dt.float32
    AF = mybir.ActivationFunctionType

    nc = bacc.Bacc(
        "TRN2", target_bir_lowering=False, debug=False,
        num_devices=NC if collectives else 1,
    )

    # ---- I/O ----
    at_d = nc.dram_tensor("at_t", [KT_NODES, ND, P, 512], f16, kind="ExternalInput")
    x0_d = nc.dram_tensor("x0", [KT_NODES, P, T], f16, kind="ExternalInput")
    w_d = {}
    w_d["w1_0"] = nc.dram_tensor("w1_0", [H // P, P, T // P, P], f16, kind="ExternalInput")
    w_d["w2_0"] = nc.dram_tensor("w2_0", [H // P, P, H // P, P], f16, kind="ExternalInput")
    for l in (1, 2):
        w_d[f"w1_{l}"] = nc.dram_tensor(f"w1_{l}", [H // P, P, H // P, P], f16, kind="ExternalInput")
    w_d["w1_3"] = nc.dram_tensor("w1_3", [O // P, P, H // P, P], f16, kind="ExternalInput")
    for l in (1, 2):
        w_d[f"w2_{l}"] = nc.dram_tensor(f"w2_{l}", [H // P, P, H // P, P], f16, kind="ExternalInput")
    whm_d = nc.dram_tensor("whm", [P, O // P, P], f16, kind="ExternalInput")
    whv_d = nc.dram_tensor("whv", [P, O // P, P], f16, kind="ExternalInput")
    b_d = {}
    for l in range(3):
        b_d[f"b1_{l}"] = nc.dram_tensor(f"b1_{l}", [P, H // P], f32, kind="ExternalInput")
    b_d["b1_3"] = nc.dram_tensor("b1_3", [P, O // P], f32, kind="ExternalInput")
    for l in range(3):
        b_d[f"b2_{l}"] = nc.dram_tensor(f"b2_{l}", [P, H // P], f32, kind="ExternalInput")
    bhm_d = nc.dram_tensor("bhm", [P, 1], f32, kind="ExternalInput")
    bhv_d = nc.dram_tensor("bhv", [P, 1], f32, kind="ExternalInput")
    eps_d = nc.dram_tensor("epst", [P, NS], f32, kind="ExternalInput")

    z_d = nc.dram_tensor("zt", [P, NS], f32, kind="ExternalOutput")
    mean_d = nc.dram_tensor("meant", [P, NS], f32, kind="ExternalOutput")
    var_d = nc.dram_tensor("vart", [P, NS], f32, kind="ExternalOutput")

    HH = H // 2
    xown = {(l, h): nc.dram_tensor(f"xown{l}_{h}", [NS, HH], f16)
            for l in (1, 2, 3) for h in (0, 1)}
    xg = {(l, h): nc.dram_tensor(f"xg{l}_{h}", [N, HH], f16, addr_space="Shared")
          for l in (1, 2, 3) for h in (0, 1)}

    rg = [list(range(NC))]

    with tile.TileContext(nc) as tc:
        with (
            tc.tile_pool(name="const", bufs=1) as const_p,
            tc.tile_pool(name="big", bufs=1) as big_p,
            tc.tile_pool(name="at", bufs=at_bufs) as at_p,
            tc.tile_pool(name="xslab", bufs=x_bufs) as x_p,
            tc.tile_pool(name="w", bufs=w_bufs) as w_p,
            tc.tile_pool(name="xo", bufs=2) as xo_p,
            tc.tile_pool(name="ps", bufs=ps_bufs, space="PSUM") as ps_p,
        ):
            ident = const_p.tile([P, P], f16, tag="ident")
            make_identity(nc, ident)

            bias_sb = {}
            for name, d in b_d.items():
                bias_sb[name] = const_p.tile(list(d.shape), f32, tag=f"b_{name}", name=f"b_{name}")
                nc.sync.dma_start(bias_sb[name][:], d[:])
            bhm_sb = const_p.tile([P, 1], f32, tag="bhm")
            nc.sync.dma_start(bhm_sb[:], bhm_d[:])
            bhv_sb = const_p.tile([P, 1], f32, tag="bhv")
            nc.sync.dma_start(bhv_sb[:], bhv_d[:])
            eps_sb = const_p.tile([P, NS], f32, tag="eps")
            nc.sync.dma_start(eps_sb[:], eps_d[:])
            whm_sb = const_p.tile([P, O // P, P], f16, tag="whm")
            nc.sync.dma_start(whm_sb[:], whm_d[:])
            whv_sb = const_p.tile([P, O // P, P], f16, tag="whv")
            nc.sync.dma_start(whv_sb[:], whv_d[:])

            def all_gather(l, h):
                if collectives:
                    nc.gpsimd.collective_compute(
                        "AllGather", mybir.AluOpType.bypass, replica_groups=rg,
                        ins=[xown[l, h][:].opt()], outs=[xg[l, h][:].opt()],
                    )
                else:
                    # sim-only stand-in: model the DMA traffic of the gather
                    for c in range(NC):
                        nc.sync.dma_start(xg[l, h][c * NS:(c + 1) * NS, :], xown[l, h][:])

            def agg(d_in, x_ap_fn, uT):
                """uT[:, mt, n*512:(n+1)*512] = sum_k x[k,m]^T @ AT[k,n]."""
                Mt = d_in // P
                for n in range(ND):
                    for g0 in range(0, Mt, agg_group):
                        gsz = min(agg_group, Mt - g0)
                        psums = [ps_p.tile([P, 512], f32, tag="mm", name=f"ps{_i}") for _i in range(gsz)]
                        for k in range(KT_NODES):
                            xs = x_p.tile([P, gsz * P], f16, tag="xslab")
                            nc.sync.dma_start(xs[:], x_ap_fn(k, g0 * P, gsz * P))
                            att = at_p.tile([P, 512], f16, tag="at")
                            nc.sync.dma_start(att[:], at_d[k, n])
                            for mi in range(gsz):
                                nc.tensor.matmul(
                                    psums[mi][:],
                                    lhsT=xs[:, mi * P:(mi + 1) * P],
                                    rhs=att[:],
                                    start=(k == 0),
                                    stop=(k == KT_NODES - 1),
                                )
                        for mi in range(gsz):
                            dst = uT[:, g0 + mi, n * 512:(n + 1) * 512]
                            if drain_split and mi % 2 == 1:
                                nc.scalar.copy(dst, psums[mi][:])
                            else:
                                nc.vector.tensor_copy(dst, psums[mi][:])

            def linear(w_dram, Kt, Mt, rhsT, outT, bias, relu, out_off=0, mts=None):
                for mt in (range(Mt) if mts is None else mts):
                    ws = w_p.tile([P, Kt, P], f16, tag="w")
                    nc.sync.dma_start(ws[:], w_dram[mt])
                    for n in range(ND):
                        p = ps_p.tile([P, 512], f32, tag="mm")
                        for k in range(Kt):
                            nc.tensor.matmul(
                                p[:],
                                lhsT=ws[:, k, :],
                                rhs=rhsT[:, k, n * 512:(n + 1) * 512],
                                start=(k == 0),
                                stop=(k == Kt - 1),
                            )
                        nc.scalar.activation(
                            outT[:, out_off + mt, n * 512:(n + 1) * 512],
                            p[:],
                            AF.Relu if relu else AF.Identity,
                            bias=bias[:, mt:mt + 1],
                        )

            def transpose_store(xT, xown_dram, half):
                mt0 = half * (H // P // 2)
                nmt = H // P // 2
                if no_transpose:
                    # sim-only: skip PE transposes, model DMA traffic directly
                    for j in range(NS // P):
                        for mt in range(nmt):
                            nc.sync.dma_start(
                                xown_dram[j * P:(j + 1) * P, mt * P:(mt + 1) * P],
                                xT[:, mt0 + mt, j * P:(j + 1) * P],
                            )
                    return
                for j in range(NS // P):
                    xo = xo_p.tile([P, nmt, P], f16, tag="xo")
                    for mt in range(nmt):
                        pt = ps_p.tile([P, P], f16, tag="mm")
                        nc.tensor.transpose(pt[:], xT[:, mt0 + mt, j * P:(j + 1) * P], ident[:])
                        if drain_split and mt % 2 == 1:
                            nc.scalar.copy(xo[:, mt, :], pt[:])
                        else:
                            nc.vector.tensor_copy(xo[:, mt, :], pt[:])
                    nc.sync.dma_start(xown_dram[j * P:(j + 1) * P, :], xo[:])

            uT0 = big_p.tile([P, T // P, NS], f16, tag="uT")
            hT = {}
            xT = {}

            # ---- layer 0 ----
            with nc.named_scope("l0_agg"):
                agg(T, lambda k, c0, w: x0_d[k, :, c0:c0 + w], uT0)
            hT[0] = big_p.tile([P, H // P, NS], f16, tag="hT", name="hT0")
            with nc.named_scope("l0_lin1"):
                linear(w_d["w1_0"], T // P, H // P, uT0, hT[0], bias_sb["b1_0"], relu=True)
            xT[0] = big_p.tile([P, H // P, NS], f16, tag="xT", name="xT0")
            for h in (0, 1):
                mts = range(h * (H // P // 2), (h + 1) * (H // P // 2))
                with nc.named_scope(f"l0_lin2_{h}"):
                    linear(w_d["w2_0"], H // P, H // P, hT[0], xT[0], bias_sb["b2_0"],
                           relu=True, mts=mts)
                with nc.named_scope(f"l0_tp_{h}"):
                    transpose_store(xT[0], xown[1, h], h)
                with nc.named_scope(f"ag1_{h}"):
                    all_gather(1, h)

            # ---- layers 1..3 ----
            for l in (1, 2, 3):
                uT = big_p.tile([P, H // P, NS], f16, tag="uT", name=f"uT{l}")
                with nc.named_scope(f"l{l}_agg"):
                    g0h, g1h = xg[l, 0], xg[l, 1]

                    def xga(k, c0, w, g0h=g0h, g1h=g1h):
                        gh = g0h if c0 < HH else g1h
                        c = c0 % HH
                        assert c + w <= HH
                        return gh[k * P:(k + 1) * P, c:c + w]

                    agg(H, xga, uT)
                mt_out = (O if l == 3 else H) // P
                hT[l] = big_p.tile([P, mt_out, NS], f16, tag="hT", name=f"hTl{l}")
                with nc.named_scope(f"l{l}_lin1"):
                    linear(w_d[f"w1_{l}"], H // P, mt_out, uT, hT[l], bias_sb[f"b1_{l}"], relu=True)
                if l < 3:
                    xT[l] = big_p.tile([P, H // P, NS], f16, tag="xT", name=f"xTl{l}")
                    for h in (0, 1):
                        mts = range(h * (H // P // 2), (h + 1) * (H // P // 2))
                        with nc.named_scope(f"l{l}_lin2_{h}"):
                            linear(w_d[f"w2_{l}"], H // P, H // P, hT[l], xT[l],
                                   bias_sb[f"b2_{l}"], relu=True, mts=mts)
                        with nc.named_scope(f"l{l}_tp_{h}"):
                            transpose_store(xT[l], xown[l + 1, h], h)
                        with nc.named_scope(f"ag{l + 1}_{h}"):
                            all_gather(l + 1, h)

            # ---- fused heads ----
            mean_sb = const_p.tile([P, NS], f32, tag="mean_sb")
            var_sb = const_p.tile([P, NS], f32, tag="var_sb")
            z_sb = const_p.tile([P, NS], f32, tag="z_sb")
            with nc.named_scope("heads"):
                for W_sb, b_sb, o_sb in ((whm_sb, bhm_sb, mean_sb), (whv_sb, bhv_sb, var_sb)):
                    for n in range(ND):
                        p = ps_p.tile([P, 512], f32, tag="mm")
                        for k in range(O // P):
                            nc.tensor.matmul(
                                p[:],
                                lhsT=W_sb[:, k, :],
                                rhs=hT[3][:, k, n * 512:(n + 1) * 512],
                                start=(k == 0),
                                stop=(k == O // P - 1),
                            )
                        nc.scalar.activation(
                            o_sb[:, n * 512:(n + 1) * 512], p[:], AF.Identity,
                            bias=b_sb[:, 0:1],
                        )
                nc.vector.tensor_tensor(z_sb[:], var_sb[:], eps_sb[:], mybir.AluOpType.mult)
                nc.vector.tensor_tensor(z_sb[:], z_sb[:], mean_sb[:], mybir.AluOpType.add)
                nc.sync.dma_start(mean_d[:], mean_sb[:])
                nc.sync.dma_start(var_d[:], var_sb[:])
                nc.sync.dma_start(z_d[:], z_sb[:])

    nc.compile()
    return nc


def _tile_lhsT(w):
    """[K, M] fp16 -> [Mt, 128, Kt, 128]; slab [mt] is SBUF-ready [128p, Kt, 128m]."""
    K, M = w.shape
    Kt, Mt = K // P, M // P
    return np.ascontiguousarray(w.reshape(Kt, P, Mt, P).transpose(2, 1, 0, 3))


def _bias_t(b):
    """[M] fp32 -> [128, Mt] (partition = feature within tile)."""
    return np.ascontiguousarray(b.reshape(-1, P).T).astype(np.float32)


def prepare_inputs(inputs):
    """Host-side preprocessing: adjacency build + layout tiling. Returns in_maps."""
    f16 = np.float16
    eeg_nodes = np.asarray(inputs["eeg_nodes"], np.float32)
    eeg_idx = np.asarray(inputs["eeg_idx"])
    src = eeg_idx[0].astype(np.int64)
    dst = eeg_idx[1].astype(np.int64)

    counts = np.bincount(src * N + dst, minlength=N * N).reshape(N, N)
    AT = counts.astype(np.float32)
    AT[np.arange(N), np.arange(N)] += 1.0  # fold GIN's (1+eps)*x self-term, eps=0
    AT16 = AT.astype(f16)
    del AT, counts

    # Activations explode to ~1.3e5 by layer 3 (> fp16 max).  Since relu is
    # positively homogeneous, scale each of layers 0-2's output by S=1/16
    # (exact power of 2), folded into w2/b2; heads unscale via x S^-3.
    S = np.float32(1.0 / 16.0)
    c = [np.float32(1.0), S, S * S, S * S * S]  # cumulative scale of x_l input

    common = {}
    common["x0"] = np.ascontiguousarray(eeg_nodes.astype(f16).reshape(KT_NODES, P, T))
    for l in range(4):
        common[f"w1_{l}"] = _tile_lhsT(np.asarray(inputs[f"w1_{l}"], np.float32).astype(f16))
        common[f"b1_{l}"] = _bias_t(np.asarray(inputs[f"b1_{l}"], np.float32) * c[l])
    for l in range(3):
        common[f"w2_{l}"] = _tile_lhsT((np.asarray(inputs[f"w2_{l}"], np.float32) * S).astype(f16))
        common[f"b2_{l}"] = _bias_t(np.asarray(inputs[f"b2_{l}"], np.float32) * c[l + 1])

    # fused heads:  mean = h3 @ (w2_3 @ wm) + (b2_3 @ wm + bm); h3 arrives
    # scaled by c[3] so the fused weight is unscaled by 1/c[3].
    w2_3 = np.asarray(inputs["w2_3"], np.float32)
    b2_3 = np.asarray(inputs["b2_3"], np.float32)
    wm = np.asarray(inputs["wm"], np.float32)
    wv = np.asarray(inputs["wv"], np.float32)
    W2m = ((w2_3 @ wm) / c[3]).astype(f16)
    W2v = ((w2_3 @ wv) / c[3]).astype(f16)
    common["whm"] = _tile_lhsT(W2m)[0]
    common["whv"] = _tile_lhsT(W2v)[0]
    common["bhm"] = (b2_3 @ wm + np.asarray(inputs["bm"], np.float32)).reshape(P, 1).astype(np.float32)
    common["bhv"] = (b2_3 @ wv + np.asarray(inputs["bv"], np.float32)).reshape(P, 1).astype(np.float32)

    eps = np.asarray(inputs["eps"], np.float32)
    in_maps = []
    for c in range(NC):
        m = dict(common)
        blk = AT16[:, c * NS:(c + 1) * NS]
        m["at_t"] = np.ascontiguousarray(
            blk.reshape(KT_NODES, P, ND, 512).transpose(0, 2, 1, 3)
        )
        m["epst"] = np.ascontiguousarray(eps[c * NS:(c + 1) * NS, :].T)
        in_maps.append(m)
    return in_maps


def get_program():
    if "nc" not in _PROGRAM_CACHE:
        _PROGRAM_CACHE["nc"] = _build_program()
    return _PROGRAM_CACHE["nc"]


def assemble_outputs(results):
    z = np.empty((N, L), np.float32)
    mean = np.empty((N, L), np.float32)
    var = np.empty((N, L), np.float32)
    for c in range(NC):
        z[c * NS:(c + 1) * NS] = results[c]["zt"].T
        mean[c * NS:(c + 1) * NS] = results[c]["meant"].T
        var[c * NS:(c + 1) * NS] = results[c]["vart"].T
    return z, mean, var


def kernel(**inputs):
    from concourse.bass_utils import run_bass_kernel_spmd

    nc = get_program()
    in_maps = prepare_inputs(inputs)
    res = run_bass_kernel_spmd(nc, in_maps, core_ids=list(range(NC)))
    return assemble_outputs(res.results)
